# revision 2
# baseline (speedup 1.0000x reference)
"""Trainium2 Bass kernel for nn_Attention_48825188221088.

  out     = lstm_out @ W.T + b        [B,S,H]
  score   = out @ out.T (per batch)   [B,S,S]
  attn    = softmax(score, -1)
  context = attn @ lstm_out           [B,S,H]

B=8, S=2048, H=1024, fp32 I/O. Sharding: data-parallel over batch B across
the 8 NeuronCores (one batch element per core); no collectives.

Per-core kernel (all matmuls bf16 on the PE with fp32 PSUM accumulation):
  1. W -> Wt (bf16, [h,o] layout) via PE transposes; b -> per-partition bias.
  2. x -> x_hi (bf16) [+ x_lo = bf16(x - x_hi) for the context matmul hi/lo
     split, recovering ~fp32 accuracy]; x_hi -> xT via PE transposes.
  3. Linear: outT[o,s] = Wt.T @ xT + b  (o on partitions, so the score
     matmul can contract over o with outT as both operands).
  4. Per 128-row q-block: score -> softmax (ACT exp with accum_out sum,
     normalization deferred) -> attnT via PE transposes -> context =
     attnT.T @ (x_hi + x_lo), scaled by 1/sum at PSUM eviction -> DMA out.
"""

import os
from contextlib import ExitStack

import numpy as np

import concourse.bass as bass
import concourse.mybir as mybir
import concourse.tile as tile
from concourse import bacc
from concourse.bass_utils import run_bass_kernel_spmd
from concourse.masks import make_identity

B, S, H = 8, 2048, 1024
P = 128  # SBUF/PSUM partitions
F = 512  # matmul free dim = one PSUM bank of fp32
SQ = S // P  # 16 s-blocks of 128
HC = H // P  # 8 h-blocks of 128
NK = S // F  # 4 score column chunks of 512
NH = H // F  # 2 context h chunks of 512

f32 = mybir.dt.float32
bf16 = mybir.dt.bfloat16

SPLIT = True  # hi/lo split for the context matmul (near-fp32 output)


def emit_iteration(nc, tc, x, W, b, out, psum, const, ident, b_sb, it=0):
    """Emit one full attention pass over a single batch element."""
    with ExitStack() as top:
        persist = top.enter_context(tc.tile_pool(name=f"persist{it}", bufs=1))

        # --- Phase W: Wt[hp, hc, o] = W[o, hc*P+hp]  (bf16) ---------------
        Wt = persist.tile([P, HC, H], bf16, name=f"Wt{it}")
        with tc.tile_pool(name=f"wstage{it}", bufs=2) as wstage:
            for oc in range(HC):
                ws = wstage.tile([P, H], f32, name="ws", tag="ws")
                nc.sync.dma_start(ws, W[oc * P : (oc + 1) * P, :])
                wb = wstage.tile([P, H], bf16, name="wb", tag="wb")
                nc.any.tensor_copy(wb, ws)
                for hc in range(HC):
                    pt = psum.tile([P, P], bf16, name="pt", tag="tr", bufs=2)
                    nc.tensor.transpose(pt, wb[:, hc * P : (hc + 1) * P], ident)
                    nc.any.tensor_copy(Wt[:, hc, oc * P : (oc + 1) * P], pt)

        # --- Phase X: x_hi/x_lo (bf16), xT[hp, hc, s] ----------------------
        x_hi = persist.tile([P, SQ, H], bf16, name=f"x_hi{it}")
        x_lo = persist.tile([P, SQ, H], bf16, name=f"x_lo{it}") if SPLIT else None
        outT = persist.tile([P, HC, S], bf16, name=f"outT{it}")

        with ExitStack() as linscope:
            xtp = linscope.enter_context(tc.tile_pool(name=f"xtp{it}", bufs=1))
            xT = xtp.tile([P, HC, S], bf16, name=f"xT{it}")
            stage = linscope.enter_context(tc.tile_pool(name=f"stage{it}", bufs=3))
            for sc in range(SQ):
                xs = stage.tile([P, H], f32, name="xs", tag="xs")
                nc.sync.dma_start(xs, x[sc * P : (sc + 1) * P, :])
                nc.any.tensor_copy(x_hi[:, sc, :], xs)
                if SPLIT:
                    nc.vector.tensor_sub(x_lo[:, sc, :], xs, x_hi[:, sc, :])
                for hc in range(HC):
                    pt = psum.tile([P, P], bf16, name="pt", tag="tr", bufs=2)
                    nc.tensor.transpose(
                        pt, x_hi[:, sc, hc * P : (hc + 1) * P], ident
                    )
                    nc.any.tensor_copy(xT[:, hc, sc * P : (sc + 1) * P], pt)

            # --- Phase L: outT[o, s] = Wt.T @ xT + b -----------------------
            for oc in range(HC):
                for ns in range(NK):
                    pl = psum.tile([P, F], f32, name="pl", tag="mm", bufs=4)
                    for hc in range(HC):
                        nc.tensor.matmul(
                            pl,
                            lhsT=Wt[:, hc, oc * P : (oc + 1) * P],
                            rhs=xT[:, hc, ns * F : (ns + 1) * F],
                            start=(hc == 0),
                            stop=(hc == HC - 1),
                        )
                    nc.vector.tensor_scalar_add(
                        outT[:, oc, ns * F : (ns + 1) * F], pl, b_sb[:, oc : oc + 1]
                    )

        # --- Phase A: per q-block score/softmax/context --------------------
        with tc.tile_pool(name=f"attn{it}", bufs=1) as ap:
            for qb in range(SQ):
                # score[q, k] for this q-block, all 2048 k columns
                sc_f32 = ap.tile([P, S], f32, name="sc_f32", tag="sc", bufs=2)
                mx = ap.tile([P, NK], f32, name="mx", tag="mx", bufs=2)
                for nk in range(NK):
                    ps = psum.tile([P, F], f32, name="ps", tag="mm", bufs=4)
                    for hc in range(HC):
                        nc.tensor.matmul(
                            ps,
                            lhsT=outT[:, hc, qb * P : (qb + 1) * P],
                            rhs=outT[:, hc, nk * F : (nk + 1) * F],
                            start=(hc == 0),
                            stop=(hc == HC - 1),
                        )
                    nc.scalar.copy(sc_f32[:, nk * F : (nk + 1) * F], ps)
                    nc.vector.reduce_max(
                        mx[:, nk : nk + 1], ps, axis=mybir.AxisListType.X
                    )
                nmx = ap.tile([P, 1], f32, name="nmx", tag="nmx", bufs=2)
                nc.vector.reduce_max(
                    nmx, mx, axis=mybir.AxisListType.X, negate=True
                )
                # attn = exp(score - max); ssum = row sum (softmax denom)
                attn_sb = ap.tile([P, S], bf16, name="attn_sb", tag="attn", bufs=2)
                ssum = ap.tile([P, 1], f32, name="ssum", tag="ssum", bufs=2)
                nc.scalar.activation(
                    attn_sb,
                    sc_f32,
                    mybir.ActivationFunctionType.Exp,
                    bias=nmx,
                    scale=1.0,
                    accum_out=ssum,
                )
                rsum = ap.tile([P, 1], f32, name="rsum", tag="rsum", bufs=2)
                nc.vector.reciprocal(rsum, ssum)
                # attnT[kp, kb, q] = attn[q, kb*P+kp]
                attnT = ap.tile([P, SQ, P], bf16, name="attnT", tag="attnT", bufs=2)
                for kb in range(SQ):
                    pt = psum.tile([P, P], bf16, name="pt", tag="tr", bufs=2)
                    nc.tensor.transpose(
                        pt, attn_sb[:, kb * P : (kb + 1) * P], ident
                    )
                    nc.any.tensor_copy(attnT[:, kb, :], pt)
                # context[q, h] = (attn @ (x_hi + x_lo)) / ssum
                ctx_sb = ap.tile([P, H], f32, name="ctx_sb", tag="ctx", bufs=2)
                for hn in range(NH):
                    pc = psum.tile([P, F], f32, name="pc", tag="ctx", bufs=2)
                    for kb in range(SQ):
                        nc.tensor.matmul(
                            pc,
                            lhsT=attnT[:, kb, :],
                            rhs=x_hi[:, kb, hn * F : (hn + 1) * F],
                            start=(kb == 0),
                            stop=(kb == SQ - 1 and not SPLIT),
                        )
                        if SPLIT:
                            nc.tensor.matmul(
                                pc,
                                lhsT=attnT[:, kb, :],
                                rhs=x_lo[:, kb, hn * F : (hn + 1) * F],
                                start=False,
                                stop=(kb == SQ - 1),
                            )
                    nc.vector.tensor_scalar_mul(
                        ctx_sb[:, hn * F : (hn + 1) * F], pc, rsum
                    )
                nc.sync.dma_start(out[qb * P : (qb + 1) * P, :], ctx_sb)


def build(n_iters=1):
    """Build the per-core Bass program. Returns compiled nc."""
    nc = bacc.Bacc("TRN2", target_bir_lowering=False, debug=False, num_devices=8)
    x = nc.dram_tensor("x", [S, H], f32, kind="ExternalInput").ap()
    W = nc.dram_tensor("W", [H, H], f32, kind="ExternalInput").ap()
    b = nc.dram_tensor("b", [H], f32, kind="ExternalInput").ap()
    out = nc.dram_tensor("ctx_out", [S, H], f32, kind="ExternalOutput").ap()

    with tile.TileContext(nc) as tc:
        with ExitStack() as top:
            const = top.enter_context(tc.tile_pool(name="const", bufs=1))
            ident = const.tile([P, P], bf16, name="ident")
            make_identity(nc, ident)
            b_sb = const.tile([P, HC], f32, name="b_sb")
            nc.sync.dma_start(b_sb, b.rearrange("(c p) -> p c", p=P))
            psum = top.enter_context(
                tc.tile_pool(name="psum", bufs=1, space="PSUM")
            )
            for it in range(n_iters):
                emit_iteration(nc, tc, x, W, b, out, psum, const, ident, b_sb, it)

    nc.compile()
    return nc


_CACHED = {}


def _get_nc(n_iters=1):
    if n_iters not in _CACHED:
        _CACHED[n_iters] = build(n_iters)
    return _CACHED[n_iters]


def kernel(lstm_out: np.ndarray, W: np.ndarray, b: np.ndarray) -> np.ndarray:
    """Full-input entry point: shards batch over 8 cores, returns [B,S,H] f32."""
    nc = _get_nc()
    lstm_out = np.ascontiguousarray(lstm_out, dtype=np.float32)
    Wc = np.ascontiguousarray(W, dtype=np.float32)
    bc = np.ascontiguousarray(b, dtype=np.float32)
    in_maps = [{"x": lstm_out[c], "W": Wc, "b": bc} for c in range(B)]
    res = run_bass_kernel_spmd(nc, in_maps, core_ids=list(range(B)))
    return np.stack([res.results[c]["ctx_out"] for c in range(B)], axis=0)


if __name__ == "__main__":
    rng = np.random.default_rng(0)
    xs = rng.standard_normal((B, S, H), dtype=np.float32)
    Ws = (rng.standard_normal((H, H), dtype=np.float32) / np.sqrt(H)).astype(
        np.float32
    )
    bs = (0.01 * rng.standard_normal(H)).astype(np.float32)
    r = kernel(xs, Ws, bs)
    print(r.shape, r.dtype)
